# revision 11
# baseline (speedup 1.0000x reference)
"""Trainium2 Bass kernel for nn_DotProcessorBlock.

Computes, for x:[B,N] f32 (B=4096, N=256), w,b:[N]:
    feat = x * w + b                      (elementwise affine on features)
    Z[b,i,j] = feat[b,i] * feat[b,j]      (batched outer product)
    out = Z.reshape(B, N*N)[:, :N*(N+1)//2]   -> [4096, 32896]

Sharding: data-parallel batch split across 8 NeuronCores (512 rows each);
w/b replicated.

The kernel is bound by HBM output-write bandwidth (~320-360 GB/s per core
sustained; 716 GB/s per stack shared with the paired core). Two traffic
reductions vs the full-f32 output (67.4 MB/core, ~211 us):

1. Symmetry dedup: Z[b] is symmetric, so of the 32896 kept entries per
   row, the strict lower triangle of the leading 128x128 block (8128) and
   the 128 tail columns (i=128, j<128) duplicate entries already present.
   The device writes only the row suffixes Z[i, j0(i):256] for i<128
   where j0(i) = i - (i%2); starting odd rows one element early keeps
   every SBUF slice 4-byte aligned with even lengths (the extra element
   is a real duplicate product). 24704 elems/row vs 32896.
2. bf16 output: the product values are written bf16 (norm rel err ~2e-3
   vs the 2e-2 gate), upcast to f32 on the host during the unshard
   gather.

Net: 25.3 MB/core, ~2.7x less HBM write traffic.

Compute: batch rows live in SBUF partitions (128-row tiles). Per output
row i one instruction. Measured per-op costs: DVE tensor_scalar (bf16
tensor + f32 per-partition scalar -> 4x packed mode) ~0.26 ns/elem +
~145 ns fixed; ACT activation-mul ~0.86 ns/elem + ~250 ns fixed. ACT
takes the 34 longest rows per tile (amortizing its fixed cost), DVE the
other 94; both land ~16-18 us/tile, just under the ~18 us/tile DMA
floor. Chunks are single-engine so a slow engine never stalls a
mostly-done chunk, and chunk DMAs are issued in estimated completion
order (the HWDGE ring pops descriptors in issue order; a not-yet-ready
chunk would head-of-line block the stream).

Startup: the SP queue spends ~8.5 us in framework preamble, so the
input loads go on the gpsimd/scalar queues which come alive ~3.5 us
earlier (x0 on gpsimd, w|b on scalar, later x tiles on gpsimd).

Host: one fancy-index gather per full row reconstructs the mirrored
columns and upcasts bf16 -> f32.
"""

from contextlib import ExitStack

import numpy as np

import concourse.bacc as bacc
import concourse.tile as tile
from concourse import mybir
from concourse.bass_utils import run_bass_kernel_spmd
from concourse.tile_rust import add_dep_helper

B_FULL = 4096
N = 256
N_CORES = 8
B_CORE = B_FULL // N_CORES          # 512
NUM_INTS = N * (N + 1) // 2         # 32896
P = 128                             # SBUF partitions = batch rows per tile
N_BT = B_CORE // P                  # 4 batch tiles per core

FP32 = mybir.dt.float32
BF16 = mybir.dt.bfloat16

# Compact row layout: for i in 0..127 store Z[i, j0(i):256] where
# j0(i) = i - (i % 2). Lengths are even and offsets stay 4B-aligned.
_J0 = [i - (i % 2) for i in range(P)]
_LEN = [N - _J0[i] for i in range(P)]
_OFF = np.concatenate([[0], np.cumsum(_LEN)]).astype(np.int64)
C_TOT = int(_OFF[P])                # 24704

# Chunk plans: (engine, row_start, row_end) in DMA-issue order, which
# approximates completion order (the HWDGE ring pops descriptors in issue
# order; a not-yet-ready chunk would head-of-line block the stream). ACT
# takes the leading (longest) rows, amortizing its ~290ns/op fixed cost;
# GPSIMD a middle block; DVE (tensor_scalar at 4x, ~0.26ns/elem +
# ~130ns/op) the rest. Each chunk is produced by a single engine.
# Tile 0 ramps with small leading DVE chunks so the output stream starts
# as early as possible (ACT's first op also pays a ~1.3us table load).
_CHUNKS0 = [
    ("D", 32, 36), ("D", 36, 44), ("D", 44, 54), ("G", 54, 66),
    ("A", 0, 16), ("D", 66, 96), ("D", 96, 128), ("A", 16, 32),
]
_CHUNKSM = [
    ("D", 32, 44), ("D", 44, 54), ("G", 54, 66), ("A", 0, 16),
    ("D", 66, 96), ("D", 96, 128), ("A", 16, 32),
]


def _check_plan(plan):
    rows = sorted(r for _, r0, r1 in plan for r in range(r0, r1))
    assert rows == list(range(P)), rows


_check_plan(_CHUNKS0)
_check_plan(_CHUNKSM)


def _emit(ctx, tc, cout, x0wb, xr):
    nc = tc.nc
    const_pool = ctx.enter_context(tc.tile_pool(name="const", bufs=1))
    x_pool = ctx.enter_context(tc.tile_pool(name="x", bufs=4))
    f_pool = ctx.enter_context(tc.tile_pool(name="feat", bufs=2))
    fb_pool = ctx.enter_context(tc.tile_pool(name="featb", bufs=2))
    o_pool = ctx.enter_context(tc.tile_pool(name="out", bufs=10))

    # Input load: all queues are gated by ~7us of framework preamble and
    # only SP/ACT/gpsimd can issue DMAs. The ACT queue issues first
    # (~7.2us vs SP's ~8.5us), so bt0's x rows + w + b arrive as ONE DMA
    # on the ACT queue ([x0 | w | b]); later x tiles load via gpsimd.
    x0wb_t = const_pool.tile([P, 3 * N], FP32, tag="x0wb")
    nc.scalar.dma_start(x0wb_t[:], x0wb[:])
    x0_t = x0wb_t[:, 0:N]
    w_t = x0wb_t[:, N:2 * N]
    b_t = x0wb_t[:, 2 * N:3 * N]

    def load_feat(bt, order_after=None):
        feat = f_pool.tile([P, N], FP32, tag="feat")
        fb16 = fb_pool.tile([P, N], BF16, tag="fb16")
        if bt == 0:
            x_t = x0_t
        else:
            x_tile = x_pool.tile([P, N], FP32, tag="x")
            nc.gpsimd.dma_start(x_tile[:], xr[(bt - 1) * P:bt * P, :])
            x_t = x_tile[:]
        mul = nc.vector.tensor_mul(feat[:], x_t, w_t)
        if order_after is not None:
            # Order-only edge: keep the next feat's DVE ops from being
            # statically scheduled ahead of the chunk-critical row ops.
            add_dep_helper(mul.ins, order_after.ins, sync=False,
                           reason="chunk rows first on DVE")
        nc.vector.tensor_add(feat[:], feat[:], b_t)
        nc.vector.tensor_copy(fb16[:], feat[:])
        return feat, fb16

    feat, fb16 = load_feat(0)
    for bt in range(N_BT):
        plan = _CHUNKS0 if bt == 0 else _CHUNKSM
        next_ld = None
        n_dve_chunks = 0
        for eng, r0, r1 in plan:
            c0 = int(_OFF[r0])
            csz = int(_OFF[r1]) - c0
            ot = o_pool.tile([P, csz], BF16, tag="ot")
            last_op = None
            for i in range(r0, r1):
                o0 = int(_OFF[i]) - c0
                if eng == "A":
                    nc.scalar.mul(ot[:, o0:o0 + _LEN[i]],
                                  feat[:, _J0[i]:N], feat[:, i:i + 1])
                elif eng == "G":
                    nc.gpsimd.tensor_scalar_mul(
                        ot[:, o0:o0 + _LEN[i]], feat[:, _J0[i]:N],
                        feat[:, i:i + 1])
                else:
                    last_op = nc.vector.tensor_scalar_mul(
                        ot[:, o0:o0 + _LEN[i]], fb16[:, _J0[i]:N],
                        feat[:, i:i + 1])
            nc.sync.dma_start(cout[bt * P:(bt + 1) * P, c0:c0 + csz],
                              ot[:, :csz])
            # Emit the next batch-tile's load+feat after the second DVE
            # chunk, ordered behind its rows on DVE.
            if eng == "D":
                n_dve_chunks += 1
                if n_dve_chunks == 2 and bt + 1 < N_BT:
                    next_ld = load_feat(bt + 1, order_after=last_op)
        if next_ld is not None:
            feat, fb16 = next_ld


def _build():
    nc = bacc.Bacc("TRN2", target_bir_lowering=False, debug=False,
                   num_devices=N_CORES)
    x0wb = nc.dram_tensor("x0wb", [P, 3 * N], FP32, kind="ExternalInput").ap()
    xr = nc.dram_tensor("xr", [B_CORE - P, N], FP32,
                        kind="ExternalInput").ap()
    cout = nc.dram_tensor("cout", [B_CORE, C_TOT], BF16,
                          kind="ExternalOutput").ap()
    with tile.TileContext(nc) as tc, ExitStack() as ctx:
        _emit(ctx, tc, cout, x0wb, xr)
    nc.compile()
    return nc


_NC_CACHE = None


def _get_nc():
    global _NC_CACHE
    if _NC_CACHE is None:
        _NC_CACHE = _build()
    return _NC_CACHE


def _build_src_index():
    """Map each of the 32896 output columns to its compact-layout column."""
    src = np.empty(NUM_INTS, np.int64)
    offs = _OFF[:P]
    j0 = np.asarray(_J0, np.int64)
    for i in range(P):
        # j >= j0(i) comes from row i itself.
        js = np.arange(_J0[i], N)
        src[i * N + _J0[i]: (i + 1) * N] = offs[i] + (js - _J0[i])
        # j < j0(i): mirror Z[i, j] = Z[j, i] from row j's suffix.
        jm = np.arange(_J0[i])
        src[i * N + jm] = offs[jm] + (i - j0[jm])
    # Tail columns (i=128, j<128): Z[128, j] = Z[j, 128].
    jm = np.arange(P)
    src[P * N: P * N + P] = offs[jm] + (P - j0[jm])
    return src


_SRC = _build_src_index()


def run(x, weight_w, weight_b, trace=False, **run_kwargs):
    x = np.ascontiguousarray(np.asarray(x, dtype=np.float32))
    w = np.asarray(weight_w, dtype=np.float32).reshape(N)
    b = np.asarray(weight_b, dtype=np.float32).reshape(N)
    assert x.shape == (B_FULL, N), x.shape

    wb = np.broadcast_to(np.concatenate([w, b]), (P, 2 * N))
    in_maps = []
    for i in range(N_CORES):
        xs = x[i * B_CORE:(i + 1) * B_CORE]
        in_maps.append({
            "x0wb": np.ascontiguousarray(np.hstack([xs[:P], wb])),
            "xr": xs[P:],
        })
    res = run_bass_kernel_spmd(
        _get_nc(), in_maps, core_ids=list(range(N_CORES)), trace=trace,
        **run_kwargs,
    )
    compact = np.concatenate([r["cout"] for r in res.results], axis=0)
    assert compact.shape == (B_FULL, C_TOT), compact.shape
    full = compact[:, _SRC].astype(np.float32)
    return full, res


def kernel(x, weight_w, weight_b):
    full, _ = run(x, weight_w, weight_b, trace=False)
    return full


# revision 14
# speedup vs baseline: 1.8948x; 1.8948x over previous
"""Trainium2 Bass kernel for nn_DotProcessorBlock.

Computes, for x:[B,N] f32 (B=4096, N=256), w,b:[N]:
    feat = x * w + b                      (elementwise affine on features)
    Z[b,i,j] = feat[b,i] * feat[b,j]      (batched outer product)
    out = Z.reshape(B, N*N)[:, :N*(N+1)//2]   -> [4096, 32896]

Sharding: data-parallel batch split across 8 NeuronCores (512 rows each);
w/b replicated.

The kernel is bound by HBM output-write bandwidth (~320-360 GB/s per core
sustained; 716 GB/s per stack shared with the paired core). Two traffic
reductions vs the full-f32 output (67.4 MB/core, ~211 us):

1. Symmetry dedup: Z[b] is symmetric, so of the 32896 kept entries per
   row, the strict lower triangle of the leading 128x128 block (8128) and
   the 128 tail columns (i=128, j<128) duplicate entries already present.
   The device writes only the row suffixes Z[i, j0(i):256] for i<128
   where j0(i) = i - (i%2); starting odd rows one element early keeps
   every SBUF slice 4-byte aligned with even lengths (the extra element
   is a real duplicate product). 24704 elems/row vs 32896.
2. bf16 output: the product values are written bf16 (norm rel err ~2e-3
   vs the 2e-2 gate), upcast to f32 on the host during the unshard
   gather.

Net: 25.3 MB/core, ~2.7x less HBM write traffic.

Compute: batch rows live in SBUF partitions (128-row tiles). Per output
row i one instruction. Measured per-op costs: DVE tensor_scalar (bf16
tensor + f32 per-partition scalar -> 4x packed mode) ~0.26 ns/elem +
~145 ns fixed; ACT activation-mul ~0.86 ns/elem + ~250 ns fixed. ACT
takes the 34 longest rows per tile (amortizing its fixed cost), DVE the
other 94; both land ~16-18 us/tile, just under the ~18 us/tile DMA
floor. Chunks are single-engine so a slow engine never stalls a
mostly-done chunk, and chunk DMAs are issued in estimated completion
order (the HWDGE ring pops descriptors in issue order; a not-yet-ready
chunk would head-of-line block the stream).

Startup: the SP queue spends ~8.5 us in framework preamble, so the
input loads go on the gpsimd/scalar queues which come alive ~3.5 us
earlier (x0 on gpsimd, w|b on scalar, later x tiles on gpsimd).

Host: one fancy-index gather per full row reconstructs the mirrored
columns and upcasts bf16 -> f32.
"""

from contextlib import ExitStack

import numpy as np

import concourse.bacc as bacc
import concourse.tile as tile
from concourse import mybir
from concourse.bass_utils import run_bass_kernel_spmd
from concourse.tile_rust import add_dep_helper

B_FULL = 4096
N = 256
N_CORES = 8
B_CORE = B_FULL // N_CORES          # 512
NUM_INTS = N * (N + 1) // 2         # 32896
P = 128                             # SBUF partitions = batch rows per tile
N_BT = B_CORE // P                  # 4 batch tiles per core

FP32 = mybir.dt.float32
BF16 = mybir.dt.bfloat16

# Compact row layout: for i in 0..127 store Z[i, j0(i):256] where
# j0(i) = i - (i % 2). Lengths are even and offsets stay 4B-aligned.
_J0 = [i - (i % 2) for i in range(P)]
_LEN = [N - _J0[i] for i in range(P)]
_OFF = np.concatenate([[0], np.cumsum(_LEN)]).astype(np.int64)
C_TOT = int(_OFF[P])                # 24704

# Chunk plans: (engine, row_start, row_end) in DMA-issue order, which
# approximates completion order (the HWDGE ring pops descriptors in issue
# order; a not-yet-ready chunk would head-of-line block the stream). ACT
# takes the leading (longest) rows, amortizing its ~290ns/op fixed cost;
# GPSIMD a middle block; DVE (tensor_scalar at 4x, ~0.26ns/elem +
# ~130ns/op) the rest. Each chunk is produced by a single engine.
# Tile 0 ramps with small leading DVE chunks so the output stream starts
# as early as possible (ACT's first op also pays a ~1.3us table load).
_CHUNKS0 = [
    ("D", 36, 40), ("D", 40, 48), ("D", 48, 60), ("D", 60, 76),
    ("A", 0, 18), ("D", 76, 100), ("D", 100, 128), ("A", 18, 36),
]
_CHUNKSM = [
    ("D", 36, 48), ("D", 48, 60), ("A", 0, 18), ("D", 60, 76),
    ("D", 76, 100), ("A", 18, 36), ("D", 100, 128),
]


def _check_plan(plan):
    rows = sorted(r for _, r0, r1 in plan for r in range(r0, r1))
    assert rows == list(range(P)), rows


_check_plan(_CHUNKS0)
_check_plan(_CHUNKSM)


def _emit(ctx, tc, cout, x0wb, xr):
    nc = tc.nc
    const_pool = ctx.enter_context(tc.tile_pool(name="const", bufs=1))
    x_pool = ctx.enter_context(tc.tile_pool(name="x", bufs=4))
    f_pool = ctx.enter_context(tc.tile_pool(name="feat", bufs=2))
    fb_pool = ctx.enter_context(tc.tile_pool(name="featb", bufs=2))
    o_pool = ctx.enter_context(tc.tile_pool(name="out", bufs=10))

    # Input load: all queues are gated by ~7us of framework preamble and
    # only SP/ACT/gpsimd can issue DMAs. The ACT queue issues first
    # (~7.2us vs SP's ~8.5us), so bt0's x rows + w + b arrive as ONE DMA
    # on the ACT queue ([x0 | w | b]); later x tiles load via gpsimd.
    x0wb_t = const_pool.tile([P, 3 * N], FP32, tag="x0wb")
    nc.scalar.dma_start(x0wb_t[:], x0wb[:])
    x0_t = x0wb_t[:, 0:N]
    w_t = x0wb_t[:, N:2 * N]
    b_t = x0wb_t[:, 2 * N:3 * N]

    def load_feat(bt, order_after=None):
        feat = f_pool.tile([P, N], FP32, tag="feat")
        fb16 = fb_pool.tile([P, N], BF16, tag="fb16")
        if bt == 0:
            x_t = x0_t
        else:
            x_tile = x_pool.tile([P, N], FP32, tag="x")
            nc.gpsimd.dma_start(x_tile[:], xr[(bt - 1) * P:bt * P, :])
            x_t = x_tile[:]
        mul = nc.vector.tensor_mul(feat[:], x_t, w_t)
        if order_after is not None:
            # Order-only edge: keep the next feat's DVE ops from being
            # statically scheduled ahead of the chunk-critical row ops.
            add_dep_helper(mul.ins, order_after.ins, sync=False,
                           reason="chunk rows first on DVE")
        nc.vector.tensor_add(feat[:], feat[:], b_t)
        nc.vector.tensor_copy(fb16[:], feat[:])
        return feat, fb16

    feat, fb16 = load_feat(0)
    for bt in range(N_BT):
        plan = _CHUNKS0 if bt == 0 else _CHUNKSM
        next_ld = None
        n_dve_chunks = 0
        for eng, r0, r1 in plan:
            c0 = int(_OFF[r0])
            csz = int(_OFF[r1]) - c0
            ot = o_pool.tile([P, csz], BF16, tag="ot")
            last_op = None
            for i in range(r0, r1):
                o0 = int(_OFF[i]) - c0
                if eng == "A":
                    nc.scalar.mul(ot[:, o0:o0 + _LEN[i]],
                                  feat[:, _J0[i]:N], feat[:, i:i + 1])
                else:
                    last_op = nc.vector.tensor_scalar_mul(
                        ot[:, o0:o0 + _LEN[i]], fb16[:, _J0[i]:N],
                        feat[:, i:i + 1])
            # Split each chunk DMA at partition 120: if the HWDGE
            # round-robin restarts per instruction, the chronically-slow
            # 16th SDMA engine (E79, ~21 GB/s vs ~26) gets 7 descriptors
            # per chunk instead of 8, trimming the straggler tail.
            PS = 120
            nc.sync.dma_start(cout[bt * P:bt * P + PS, c0:c0 + csz],
                              ot[0:PS, :csz])
            nc.sync.dma_start(cout[bt * P + PS:(bt + 1) * P, c0:c0 + csz],
                              ot[PS:P, :csz])
            # Emit the next batch-tile's load+feat after the second DVE
            # chunk, ordered behind its rows on DVE.
            if eng == "D":
                n_dve_chunks += 1
                if n_dve_chunks == 2 and bt + 1 < N_BT:
                    next_ld = load_feat(bt + 1, order_after=last_op)
        if next_ld is not None:
            feat, fb16 = next_ld


def _build():
    nc = bacc.Bacc("TRN2", target_bir_lowering=False, debug=False,
                   num_devices=N_CORES)
    x0wb = nc.dram_tensor("x0wb", [P, 3 * N], FP32, kind="ExternalInput").ap()
    xr = nc.dram_tensor("xr", [B_CORE - P, N], FP32,
                        kind="ExternalInput").ap()
    cout = nc.dram_tensor("cout", [B_CORE, C_TOT], BF16,
                          kind="ExternalOutput").ap()
    with tile.TileContext(nc) as tc, ExitStack() as ctx:
        _emit(ctx, tc, cout, x0wb, xr)
    nc.compile()
    return nc


_NC_CACHE = None


def _get_nc():
    global _NC_CACHE
    if _NC_CACHE is None:
        _NC_CACHE = _build()
    return _NC_CACHE


def _build_src_index():
    """Map each of the 32896 output columns to its compact-layout column."""
    src = np.empty(NUM_INTS, np.int64)
    offs = _OFF[:P]
    j0 = np.asarray(_J0, np.int64)
    for i in range(P):
        # j >= j0(i) comes from row i itself.
        js = np.arange(_J0[i], N)
        src[i * N + _J0[i]: (i + 1) * N] = offs[i] + (js - _J0[i])
        # j < j0(i): mirror Z[i, j] = Z[j, i] from row j's suffix.
        jm = np.arange(_J0[i])
        src[i * N + jm] = offs[jm] + (i - j0[jm])
    # Tail columns (i=128, j<128): Z[128, j] = Z[j, 128].
    jm = np.arange(P)
    src[P * N: P * N + P] = offs[jm] + (P - j0[jm])
    return src


_SRC = _build_src_index()


def run(x, weight_w, weight_b, trace=False, **run_kwargs):
    x = np.ascontiguousarray(np.asarray(x, dtype=np.float32))
    w = np.asarray(weight_w, dtype=np.float32).reshape(N)
    b = np.asarray(weight_b, dtype=np.float32).reshape(N)
    assert x.shape == (B_FULL, N), x.shape

    wb = np.broadcast_to(np.concatenate([w, b]), (P, 2 * N))
    in_maps = []
    for i in range(N_CORES):
        xs = x[i * B_CORE:(i + 1) * B_CORE]
        in_maps.append({
            "x0wb": np.ascontiguousarray(np.hstack([xs[:P], wb])),
            "xr": xs[P:],
        })
    res = run_bass_kernel_spmd(
        _get_nc(), in_maps, core_ids=list(range(N_CORES)), trace=trace,
        **run_kwargs,
    )
    compact = np.concatenate([r["cout"] for r in res.results], axis=0)
    assert compact.shape == (B_FULL, C_TOT), compact.shape
    full = compact[:, _SRC].astype(np.float32)
    return full, res


def kernel(x, weight_w, weight_b):
    full, _ = run(x, weight_w, weight_b, trace=False)
    return full


# revision 15
# speedup vs baseline: 2.3369x; 1.2333x over previous
"""Trainium2 Bass kernel for nn_DotProcessorBlock.

Computes, for x:[B,N] f32 (B=4096, N=256), w,b:[N]:
    feat = x * w + b                      (elementwise affine on features)
    Z[b,i,j] = feat[b,i] * feat[b,j]      (batched outer product)
    out = Z.reshape(B, N*N)[:, :N*(N+1)//2]   -> [4096, 32896]

Sharding: data-parallel batch split across 8 NeuronCores (512 rows each);
w/b replicated.

The kernel is bound by HBM output-write bandwidth (~320-360 GB/s per core
sustained; 716 GB/s per stack shared with the paired core). Two traffic
reductions vs the full-f32 output (67.4 MB/core, ~211 us):

1. Symmetry dedup: Z[b] is symmetric, so of the 32896 kept entries per
   row, the strict lower triangle of the leading 128x128 block (8128) and
   the 128 tail columns (i=128, j<128) duplicate entries already present.
   The device writes only the row suffixes Z[i, j0(i):256] for i<128
   where j0(i) = i - (i%2); starting odd rows one element early keeps
   every SBUF slice 4-byte aligned with even lengths (the extra element
   is a real duplicate product). 24704 elems/row vs 32896.
2. bf16 output: the product values are written bf16 (norm rel err ~2e-3
   vs the 2e-2 gate), upcast to f32 on the host during the unshard
   gather.

Net: 25.3 MB/core, ~2.7x less HBM write traffic.

Compute: batch rows live in SBUF partitions (128-row tiles). Per output
row i one instruction. Measured per-op costs: DVE tensor_scalar (bf16
tensor + f32 per-partition scalar -> 4x packed mode) ~0.26 ns/elem +
~145 ns fixed; ACT activation-mul ~0.86 ns/elem + ~250 ns fixed. ACT
takes the 34 longest rows per tile (amortizing its fixed cost), DVE the
other 94; both land ~16-18 us/tile, just under the ~18 us/tile DMA
floor. Chunks are single-engine so a slow engine never stalls a
mostly-done chunk, and chunk DMAs are issued in estimated completion
order (the HWDGE ring pops descriptors in issue order; a not-yet-ready
chunk would head-of-line block the stream).

Startup: the SP queue spends ~8.5 us in framework preamble, so the
input loads go on the gpsimd/scalar queues which come alive ~3.5 us
earlier (x0 on gpsimd, w|b on scalar, later x tiles on gpsimd).

Host: one fancy-index gather per full row reconstructs the mirrored
columns and upcasts bf16 -> f32.
"""

from contextlib import ExitStack

import numpy as np

import concourse.bacc as bacc
import concourse.tile as tile
from concourse import mybir
from concourse.bass_utils import run_bass_kernel_spmd
from concourse.tile_rust import add_dep_helper

B_FULL = 4096
N = 256
N_CORES = 8
B_CORE = B_FULL // N_CORES          # 512
NUM_INTS = N * (N + 1) // 2         # 32896
P = 128                             # SBUF partitions = batch rows per tile
N_BT = B_CORE // P                  # 4 batch tiles per core

FP32 = mybir.dt.float32
BF16 = mybir.dt.bfloat16

# Compact row layout: for i in 0..127 store Z[i, j0(i):256] where
# j0(i) = i - (i % 2). Lengths are even and offsets stay 4B-aligned.
_J0 = [i - (i % 2) for i in range(P)]
_LEN = [N - _J0[i] for i in range(P)]
_OFF = np.concatenate([[0], np.cumsum(_LEN)]).astype(np.int64)
C_TOT = int(_OFF[P])                # 24704

# Chunk plans: (engine, row_start, row_end) in DMA-issue order, which
# approximates completion order (the HWDGE ring pops descriptors in issue
# order; a not-yet-ready chunk would head-of-line block the stream). ACT
# takes the leading (longest) rows, amortizing its ~290ns/op fixed cost;
# GPSIMD a middle block; DVE (tensor_scalar at 4x, ~0.26ns/elem +
# ~130ns/op) the rest. Each chunk is produced by a single engine.
# Tile 0 ramps with small leading DVE chunks so the output stream starts
# as early as possible (ACT's first op also pays a ~1.3us table load).
_CHUNKS0 = [
    ("D", 36, 40), ("D", 40, 48), ("D", 48, 60), ("D", 60, 76),
    ("A", 0, 18), ("D", 76, 100), ("D", 100, 128), ("A", 18, 36),
]
_CHUNKSM = [
    ("D", 36, 48), ("D", 48, 60), ("A", 0, 18), ("D", 60, 76),
    ("D", 76, 100), ("A", 18, 36), ("D", 100, 128),
]


def _check_plan(plan):
    rows = sorted(r for _, r0, r1 in plan for r in range(r0, r1))
    assert rows == list(range(P)), rows


_check_plan(_CHUNKS0)
_check_plan(_CHUNKSM)


def _emit(ctx, tc, cout, x0wb, xr):
    nc = tc.nc
    const_pool = ctx.enter_context(tc.tile_pool(name="const", bufs=1))
    x_pool = ctx.enter_context(tc.tile_pool(name="x", bufs=4))
    f_pool = ctx.enter_context(tc.tile_pool(name="feat", bufs=2))
    fb_pool = ctx.enter_context(tc.tile_pool(name="featb", bufs=2))
    o_pool = ctx.enter_context(tc.tile_pool(name="out", bufs=10))

    # Input load: all queues are gated by ~7us of framework preamble and
    # only SP/ACT/gpsimd can issue DMAs. The ACT queue issues first
    # (~7.2us vs SP's ~8.5us), so bt0's x rows + w + b arrive as ONE DMA
    # on the ACT queue ([x0 | w | b]); later x tiles load via gpsimd.
    x0wb_t = const_pool.tile([P, 3 * N], FP32, tag="x0wb")
    nc.scalar.dma_start(x0wb_t[:], x0wb[:])
    x0_t = x0wb_t[:, 0:N]
    w_t = x0wb_t[:, N:2 * N]
    b_t = x0wb_t[:, 2 * N:3 * N]

    def load_feat(bt, order_after=None):
        feat = f_pool.tile([P, N], FP32, tag="feat")
        fb16 = fb_pool.tile([P, N], BF16, tag="fb16")
        if bt == 0:
            x_t = x0_t
        else:
            x_tile = x_pool.tile([P, N], FP32, tag="x")
            nc.gpsimd.dma_start(x_tile[:], xr[(bt - 1) * P:bt * P, :])
            x_t = x_tile[:]
        mul = nc.vector.tensor_mul(feat[:], x_t, w_t)
        if order_after is not None:
            # Order-only edge: keep the next feat's DVE ops from being
            # statically scheduled ahead of the chunk-critical row ops.
            add_dep_helper(mul.ins, order_after.ins, sync=False,
                           reason="chunk rows first on DVE")
        nc.vector.tensor_add(feat[:], feat[:], b_t)
        nc.vector.tensor_copy(fb16[:], feat[:])
        return feat, fb16

    feat, fb16 = load_feat(0)
    for bt in range(N_BT):
        plan = _CHUNKS0 if bt == 0 else _CHUNKSM
        next_ld = None
        n_dve_chunks = 0
        for eng, r0, r1 in plan:
            c0 = int(_OFF[r0])
            csz = int(_OFF[r1]) - c0
            ot = o_pool.tile([P, csz], BF16, tag="ot")
            last_op = None
            for i in range(r0, r1):
                o0 = int(_OFF[i]) - c0
                if eng == "A":
                    nc.scalar.mul(ot[:, o0:o0 + _LEN[i]],
                                  feat[:, _J0[i]:N], feat[:, i:i + 1])
                else:
                    last_op = nc.vector.tensor_scalar_mul(
                        ot[:, o0:o0 + _LEN[i]], fb16[:, _J0[i]:N],
                        feat[:, i:i + 1])
            nc.sync.dma_start(cout[bt * P:(bt + 1) * P, c0:c0 + csz],
                              ot[:, :csz])
            # Emit the next batch-tile's load+feat after the second DVE
            # chunk, ordered behind its rows on DVE.
            if eng == "D":
                n_dve_chunks += 1
                if n_dve_chunks == 2 and bt + 1 < N_BT:
                    next_ld = load_feat(bt + 1, order_after=last_op)
        if next_ld is not None:
            feat, fb16 = next_ld


def _build():
    nc = bacc.Bacc("TRN2", target_bir_lowering=False, debug=False,
                   num_devices=N_CORES)
    x0wb = nc.dram_tensor("x0wb", [P, 3 * N], FP32, kind="ExternalInput").ap()
    xr = nc.dram_tensor("xr", [B_CORE - P, N], FP32,
                        kind="ExternalInput").ap()
    cout = nc.dram_tensor("cout", [B_CORE, C_TOT], BF16,
                          kind="ExternalOutput").ap()
    with tile.TileContext(nc) as tc, ExitStack() as ctx:
        _emit(ctx, tc, cout, x0wb, xr)
    nc.compile()
    return nc


_NC_CACHE = None


def _get_nc():
    global _NC_CACHE
    if _NC_CACHE is None:
        _NC_CACHE = _build()
    return _NC_CACHE


def _build_src_index():
    """Map each of the 32896 output columns to its compact-layout column."""
    src = np.empty(NUM_INTS, np.int64)
    offs = _OFF[:P]
    j0 = np.asarray(_J0, np.int64)
    for i in range(P):
        # j >= j0(i) comes from row i itself.
        js = np.arange(_J0[i], N)
        src[i * N + _J0[i]: (i + 1) * N] = offs[i] + (js - _J0[i])
        # j < j0(i): mirror Z[i, j] = Z[j, i] from row j's suffix.
        jm = np.arange(_J0[i])
        src[i * N + jm] = offs[jm] + (i - j0[jm])
    # Tail columns (i=128, j<128): Z[128, j] = Z[j, 128].
    jm = np.arange(P)
    src[P * N: P * N + P] = offs[jm] + (P - j0[jm])
    return src


_SRC = _build_src_index()


def run(x, weight_w, weight_b, trace=False, **run_kwargs):
    x = np.ascontiguousarray(np.asarray(x, dtype=np.float32))
    w = np.asarray(weight_w, dtype=np.float32).reshape(N)
    b = np.asarray(weight_b, dtype=np.float32).reshape(N)
    assert x.shape == (B_FULL, N), x.shape

    wb = np.broadcast_to(np.concatenate([w, b]), (P, 2 * N))
    in_maps = []
    for i in range(N_CORES):
        xs = x[i * B_CORE:(i + 1) * B_CORE]
        in_maps.append({
            "x0wb": np.ascontiguousarray(np.hstack([xs[:P], wb])),
            "xr": xs[P:],
        })
    res = run_bass_kernel_spmd(
        _get_nc(), in_maps, core_ids=list(range(N_CORES)), trace=trace,
        **run_kwargs,
    )
    compact = np.concatenate([r["cout"] for r in res.results], axis=0)
    assert compact.shape == (B_FULL, C_TOT), compact.shape
    full = compact[:, _SRC].astype(np.float32)
    return full, res


def kernel(x, weight_w, weight_b):
    full, _ = run(x, weight_w, weight_b, trace=False)
    return full
